# revision 15
# baseline (speedup 1.0000x reference)
"""Distortion-loss (eff_distloss) Bass kernel for Trainium2, 8 NeuronCores.

Inputs (full): weights/distances/intervals, each [262144, 128] f32.
Output: scalar f32 loss.

Math: per ray (w, m, s in R^128):
  uni = sum_j s_j w_j^2
  bi  = sum_{j>k} w_j w_k (m_j - m_k) = wm^T (SL - SU) w,  wm = w*m,
        SL/SU strictly lower/upper triangular ones.
  loss = 0.01 * mean_rays(uni/3 + 2*bi)

Total bi over a batch of rays = <A, G1>_F with A = SL - SU (constant) and
G1 = WM^T @ W accumulated over all rays (one 128x128 matmul per 128 rays,
accumulated in PSUM). Likewise uni = sum diag(G2), G2 = SW^T @ W with
sw = s*w. The O(N) cumsum formulation is never materialized.

Sharding: pure data-parallel over the ray axis, B=262144 -> 32768 rays on
each of the 8 cores. Each core returns 128+128 per-partition partial sums
(bi and uni); the host does the final tiny reduction and scaling.

Raw-bass implementation (no Tile): three engine programs (sync=DMA,
vector=elementwise bf16 products, tensor=Gram matmuls) with explicit
semaphores and NB-deep ring buffers.
"""

import numpy as np

import concourse.bass as bass
import concourse.mybir as mybir
from concourse.bass_utils import run_bass_kernel_spmd

B, N = 262144, 128
NCORES = 8
B_PER = B // NCORES  # 32768 rays per core
P = 128  # SBUF partitions = rays per matmul block
R = 16  # rays per partition per tile
T = B_PER // (P * R)  # 16 tiles per core
FREE = R * N  # 2048 f32 per partition per tile
NB = 3  # ring depth

F32 = mybir.dt.float32
BF16 = mybir.dt.bfloat16

LOSS_WEIGHT = 0.01

_cached = {}


def _build_nc() -> bass.Bass:
    nc = bass.Bass(trn_type="TRN2")

    w_h = nc.declare_dram_parameter("weights", [B_PER, N], F32, isOutput=False)
    m_h = nc.declare_dram_parameter("distances", [B_PER, N], F32, isOutput=False)
    s_h = nc.declare_dram_parameter("intervals", [B_PER, N], F32, isOutput=False)
    a2_h = nc.declare_dram_parameter("a2mat", [P, N], F32, isOutput=False)
    id_h = nc.declare_dram_parameter("imat", [P, N], F32, isOutput=False)
    out_h = nc.declare_dram_parameter("partials", [P, 2], F32, isOutput=True)

    w_r = w_h[:, :].rearrange("(t p r) n -> t p (r n)", t=T, p=P, r=R)
    m_r = m_h[:, :].rearrange("(t p r) n -> t p (r n)", t=T, p=P, r=R)
    s_r = s_h[:, :].rearrange("(t p r) n -> t p (r n)", t=T, p=P, r=R)

    with (
        nc.sbuf_tensor([P, NB * FREE], F32) as w_sb,
        nc.sbuf_tensor([P, NB * FREE], F32) as m_sb,
        nc.sbuf_tensor([P, NB * FREE], F32) as s_sb,
        nc.sbuf_tensor([P, NB * FREE], BF16) as wm_sb,
        nc.sbuf_tensor([P, NB * FREE], BF16) as sw_sb,
        nc.sbuf_tensor([P, NB * FREE], BF16) as wb_sb,
        nc.sbuf_tensor([P, N], F32) as a2_sb,
        nc.sbuf_tensor([P, N], F32) as id_sb,
        nc.sbuf_tensor([P, 2], F32) as out_sb,
        nc.sbuf_tensor([P, N], F32) as tr_sb,
        nc.psum_tensor([P, N], F32) as g1_ps,
        nc.psum_tensor([P, N], F32) as g2_ps,
        nc.semaphore("dma_c") as dma_c,
        nc.semaphore("dma_sem") as dma_sem,
        nc.semaphore("dve_sem") as dve_sem,
        nc.semaphore("pe_sem") as pe_sem,
        nc.semaphore("fin_sem") as fin_sem,
        nc.Block() as block,
    ):

        def sl(t):
            return slice((t % NB) * FREE, (t % NB + 1) * FREE)

        @block.sync
        def _(sync: bass.BassEngine):
            sync.dma_start(out=a2_sb[:], in_=a2_h[:, :]).then_inc(dma_c, 16)
            sync.dma_start(out=id_sb[:], in_=id_h[:, :]).then_inc(dma_c, 16)
            for t in range(T):
                if t >= NB:
                    # io ring slot (t-NB) fully consumed by DVE
                    sync.wait_ge(dve_sem, t - NB + 1)
                sync.dma_start(out=w_sb[:, sl(t)], in_=w_r[t]).then_inc(dma_sem, 16)
                sync.dma_start(out=m_sb[:, sl(t)], in_=m_r[t]).then_inc(dma_sem, 16)
                sync.dma_start(out=s_sb[:, sl(t)], in_=s_r[t]).then_inc(dma_sem, 16)
            sync.wait_ge(fin_sem, 1)
            sync.dma_start(out=out_h[:, :], in_=out_sb[:]).then_inc(dma_sem, 16)
            sync.wait_ge(dma_sem, 16 * (3 * T + 1))

        @block.vector
        def _(vector: bass.BassEngine):
            for t in range(T):
                vector.wait_ge(dma_sem, 48 * (t + 1))
                if t >= NB:
                    # bf16 ring slot (t-NB) fully consumed by PE
                    vector.wait_ge(pe_sem, t - NB + 1)
                vector.tensor_mul(wm_sb[:, sl(t)], w_sb[:, sl(t)], m_sb[:, sl(t)])
                vector.tensor_mul(sw_sb[:, sl(t)], s_sb[:, sl(t)], w_sb[:, sl(t)])
                vector.tensor_copy(out=wb_sb[:, sl(t)], in_=w_sb[:, sl(t)]).then_inc(
                    dve_sem, 1
                )
            vector.wait_ge(pe_sem, T)
            vector.wait_ge(dma_c, 32)
            vector.tensor_mul(tr_sb[:], g1_ps[:], a2_sb[:])
            vector.tensor_reduce(
                out_sb[:, 0:1],
                tr_sb[:],
                axis=mybir.AxisListType.X,
                op=mybir.AluOpType.add,
            )
            vector.tensor_mul(tr_sb[:], g2_ps[:], id_sb[:])
            vector.tensor_reduce(
                out_sb[:, 1:2],
                tr_sb[:],
                axis=mybir.AxisListType.X,
                op=mybir.AluOpType.add,
            ).then_inc(fin_sem, 1)

        @block.tensor
        def _(tensor: bass.BassEngine):
            for t in range(T):
                tensor.wait_ge(dve_sem, t + 1)
                base = (t % NB) * FREE
                last_mm = None
                for r in range(R):
                    blk = slice(base + r * N, base + (r + 1) * N)
                    first = t == 0 and r == 0
                    last = t == T - 1 and r == R - 1
                    nc.tensor.matmul(
                        out=g1_ps[:],
                        lhsT=wm_sb[:, blk],
                        rhs=wb_sb[:, blk],
                        start=first,
                        stop=last,
                    )
                    last_mm = nc.tensor.matmul(
                        out=g2_ps[:],
                        lhsT=sw_sb[:, blk],
                        rhs=wb_sb[:, blk],
                        start=first,
                        stop=last,
                    )
                last_mm.then_inc(pe_sem, 1)

    return nc


def _a2mat() -> np.ndarray:
    a = np.tril(np.ones((N, N), np.float32), -1) - np.triu(
        np.ones((N, N), np.float32), 1
    )
    return np.ascontiguousarray(a, dtype=np.float32)


def _imat() -> np.ndarray:
    return np.ascontiguousarray(np.eye(N, dtype=np.float32))


def kernel(weights: np.ndarray, distances: np.ndarray, intervals: np.ndarray):
    if "nc" not in _cached:
        _cached["nc"] = _build_nc()
    nc = _cached["nc"]

    w8 = np.ascontiguousarray(weights, np.float32).reshape(NCORES, B_PER, N)
    m8 = np.ascontiguousarray(distances, np.float32).reshape(NCORES, B_PER, N)
    s8 = np.ascontiguousarray(intervals, np.float32).reshape(NCORES, B_PER, N)
    a2 = _a2mat()
    im = _imat()

    in_maps = [
        {
            "weights": w8[i],
            "distances": m8[i],
            "intervals": s8[i],
            "a2mat": a2,
            "imat": im,
        }
        for i in range(NCORES)
    ]
    res = run_bass_kernel_spmd(nc, in_maps, list(range(NCORES))).results

    total_bi = 0.0
    total_uni = 0.0
    for i in range(NCORES):
        p = res[i]["partials"].astype(np.float64)
        total_bi += p[:, 0].sum()
        total_uni += p[:, 1].sum()

    loss = LOSS_WEIGHT * ((total_uni / 3.0) + 2.0 * total_bi) / B
    return np.asarray(loss, dtype=np.float32)
